# revision 8
# baseline (speedup 1.0000x reference)
"""Pairwise-interaction kernel for Trainium2 (raw Bass), 8-core SPMD.

Computes out[b, p, :] = x[b, i(p), :] * x[b, j(p), :] for all pairs
(i < j) of the F=26 feature rows, p ordered row-major (i outer, j inner).

Sharding: data-parallel over the batch dim (16384 -> 8 x 2048), no
cross-core communication.

Design (arrived at over v1..v5; v1 f32 baseline was ~221us, this is
~108us, essentially the DMA-fabric roofline for the mandatory traffic):
  * All tensors bf16: DVE 2x packing mode doubles tensor_tensor
    throughput vs f32 (the f32 kernel was vector-bound at ~203us busy)
    AND halves HBM traffic to 42.6MB stores + 3.4MB loads per core.
    The added rounding error (two input roundings + one output
    rounding, each <=2^-8: ~1.2% worst case, 1.07e-2 measured) is well
    inside the 2e-2 relative-error gate; f32<->bf16 conversion happens
    on the host.
  * Samples are interleaved G=4 per partition row (sample =
    t*P*G + p*G + g): every TT instruction covers all 4 groups through
    a [P, G, nrep, D] broadcast AP (amortizes the ~58-cycle DVE
    per-instruction bubble -> DVE ~95us busy), and each DMA descriptor
    row is a multi-KB contiguous DRAM run (4 consecutive samples per
    partition). v2 measured that ~5KB descriptor rows are
    packet-overhead-bound at ~338 GB/s; with 10-17KB rows the stores
    sustain ~420 GB/s, ~97% of the 435 GB/s SBUF-AXI fabric ceiling.
  * The exec floor is the store stream: first-chunk-ready +
    42.6MB / ~420GB/s. The 16 SDMA engines are shared by both HWDGE
    rings, so splitting stores across rings buys nothing; all stores
    ride the sync ring, and loads ride the scalar ring so a load never
    queues behind a multi-MB store (v3 lost 23us to exactly that).
    Store chunks are pair-ranges sized tiny/huge/small (25/264/36
    pairs) so the stream starts ~2us into the first sweep and the
    post-compute drain is only ~2.8us.
  * All NTS=4 input loads are issued up-front (XB=NTS buffers, 27KB of
    SBUF) with no slot-reuse waits.

Raw-Bass sync scheme (one semaphore wait per instruction; extra
ordering uses standalone wait_ge ops on the engine queue):
  sem_ld (+16 per load DMA, scalar ring)
  sem_st (+16 per store DMA, sync ring; NCH chunk-stores per supertile)
  sem_tt (+1 by the last TT of each chunk, vector engine)
"""

import numpy as np
import ml_dtypes

import concourse.bass as bass
from concourse import mybir
from concourse.bass_utils import run_bass_kernel_spmd

B, F, D = 16384, 26, 32
NCORES = 8
BC = B // NCORES           # 2048 samples per core
P = 128                    # SBUF partitions
G = 4                      # sample groups per supertile (consecutive rows)
NTS = BC // (P * G)        # 4 supertiles per core
FD = F * D                 # 832
NPAIR = F * (F - 1) // 2   # 325
OD = NPAIR * D             # 10400

XB = NTS                   # all input supertiles resident at once
YB = 2                     # output supertile buffers

# i-block ranges per store chunk: pair counts (25, 264, 36) - tiny
# first chunk so the store stream starts early, one huge middle chunk
# (16.9KB descriptor rows sustain peak DMA rate), small last chunk so
# the post-compute drain is short.
CHUNKS = [(0, 1), (1, 17), (17, 25)]
NCH = len(CHUNKS)

BF16 = mybir.dt.bfloat16
NP_BF16 = ml_dtypes.bfloat16


def _pair_off(i_lo):
    # first output pair index for block i = i_lo
    return sum(F - 1 - i for i in range(i_lo))


_nc_cache = None


def _build_nc():
    nc = bass.Bass()
    x = nc.declare_dram_parameter("x", [BC, FD], BF16, isOutput=False)
    y = nc.declare_dram_parameter("y", [BC, OD], BF16, isOutput=True)
    # sample s = t*P*G + p*G + g: partition p's G samples are consecutive
    # DRAM rows, so per-partition DMA runs are long and contiguous.
    xv = x[:].rearrange("(t p g) m -> t p (g m)", p=P, g=G)
    yv = y[:].rearrange("(t p g) m -> t p g m", p=P, g=G)

    with (
        nc.sbuf_tensor([P, XB * G * FD], BF16) as xbuf,
        nc.sbuf_tensor([P, YB * G * OD], BF16) as ybuf,
        nc.semaphore("sem_ld") as sem_ld,
        nc.semaphore("sem_st") as sem_st,
        nc.semaphore("sem_tt") as sem_tt,
        nc.Block() as blk,
    ):
        xts = [xbuf[:, b * G * FD : (b + 1) * G * FD] for b in range(XB)]
        yts = [ybuf[:, b * G * OD : (b + 1) * G * OD] for b in range(YB)]

        @blk.scalar
        def _(scalar):
            for t in range(NTS):
                scalar.dma_start(xts[t], xv[t]).then_inc(sem_ld, 16)

        @blk.sync
        def _(sync):
            for t in range(NTS):
                yt = yts[t % YB].rearrange("p (g m) -> p g m", g=G)
                for c, (i_lo, i_hi) in enumerate(CHUNKS):
                    p_lo, p_hi = _pair_off(i_lo), _pair_off(i_hi)
                    st = sync.dma_start(
                        yv[t][:, :, p_lo * D : p_hi * D],
                        yt[:, :, p_lo * D : p_hi * D],
                    )
                    st._wait_ge(sem_tt, NCH * t + c + 1)
                    st.then_inc(sem_st, 16)

        @blk.vector
        def _(v):
            for t in range(NTS):
                xt = xts[t].rearrange("p (g m) -> p g m", g=G)
                yt = yts[t % YB].rearrange("p (g m) -> p g m", g=G)
                v.wait_ge(sem_ld, 16 * (t + 1))
                for c, (i_lo, i_hi) in enumerate(CHUNKS):
                    if t >= YB:
                        # chunk c of ybuf slot t-YB has been stored
                        v.wait_ge(sem_st, 16 * (NCH * (t - YB) + c + 1))
                    off = _pair_off(i_lo)
                    for i in range(i_lo, i_hi):
                        nrep = F - 1 - i
                        in0 = (
                            xt[:, :, i * D : (i + 1) * D]
                            .unsqueeze(2)
                            .broadcast_to([P, G, nrep, D])
                        )
                        in1 = xt[:, :, (i + 1) * D : FD].rearrange(
                            "p g (r d) -> p g r d", d=D
                        )
                        outap = yt[
                            :, :, off * D : (off + nrep) * D
                        ].rearrange("p g (r d) -> p g r d", d=D)
                        tt = nc.vector.tensor_mul(outap, in0, in1)
                        off += nrep
                    tt.then_inc(sem_tt, 1)

    return nc


def _make_in_maps(inputs: np.ndarray):
    x = np.asarray(inputs, dtype=np.float32).reshape(B, FD).astype(NP_BF16)
    shards = np.ascontiguousarray(x.reshape(NCORES, BC, FD))
    return [{"x": shards[c]} for c in range(NCORES)]


def kernel(inputs: np.ndarray) -> np.ndarray:
    global _nc_cache
    if _nc_cache is None:
        _nc_cache = _build_nc()
    nc = _nc_cache

    in_maps = _make_in_maps(inputs)
    res = run_bass_kernel_spmd(nc, in_maps, list(range(NCORES)))
    out = np.concatenate([res.results[c]["y"] for c in range(NCORES)], axis=0)
    return out.astype(np.float32).reshape(B, NPAIR, D)
